# revision 1
# baseline (speedup 1.0000x reference)
"""MoE actor kernel for Trainium2 (8 NeuronCores, data-parallel).

For x [B, 512]: gate = sparse top-2 softmax over 16 experts (router Wg);
mean = sum_e gate[b,e] (x @ Wm[e].T + bm[e]);
log_std = affine(tanh(sum_e gate[b,e] (x @ Ws[e].T + bs[e]))).

Per core (BL = 8192 tokens): split-bf16 router GEMM (x and Wg split into
hi+lo bf16 so top-2 selection matches the f32 reference) -> softmax/top2
-> counting sort by expert via PE cumsum matmuls into fixed-capacity
expert slots -> permutation inversion via SBUF scatter-add (carries the
gates too) -> per-expert gather-transpose dispatch -> bf16 GEMM (mean|std
concatenated, bias folded) -> dma transpose -> gate scale -> scatter-add
combine -> tanh postprocess.
"""

import numpy as np

LOG_STD_MIN = -5.0
LOG_STD_MAX = 2.0

B, OBS, ACT, E, TOPK = 65536, 512, 64, 16, 2
NCORES = 8
BL = B // NCORES
CAP = 1536

_COMPILED = {}


def build(nc, mybir, tile, BL, CAP):
    import contextlib
    import os
    STOP = os.environ.get("KSTOP", "")
    REPS = int(os.environ.get("KREPS", "1"))

    class _StopBuild(Exception):
        pass

    f32 = mybir.dt.float32
    bf16 = mybir.dt.bfloat16
    i16 = mybir.dt.int16

    NT = BL // 128            # token tiles
    NPAIR = 2 * BL
    NPT = 2 * NT              # pair tiles (<= 128)
    NPOS = E * CAP            # padded positions
    NPC = NPOS // 16          # idx cols for positions
    HC = CAP // 2             # half-expert gather chunk
    HCC = HC // 16
    NSLOT = NPOS // 128
    NGRP = NSLOT // 2         # parity groups for inversion scratch
    RC = min(512, BL)         # router chunk tokens
    NRC = BL // RC
    OG = NT // 2 + 1          # combine groups (+1 trash)
    SCH = min(2048, NPAIR)    # inversion scatter chunk (pairs)
    EW = NPT * E              # excl width
    SW = min(512, EW)
    FR = 16 if NT % 32 == 0 else 2  # finalize rounds
    HT = NT // FR
    assert NPT <= 128 and EW % SW == 0 and NT % FR == 0 and (NT // FR) % 2 == 0
    assert HC % 128 == 0

    AX = mybir.AxisListType.X
    alu = mybir.AluOpType
    act_t = mybir.ActivationFunctionType

    xd = nc.declare_dram_parameter("x", [BL, OBS], f32, isOutput=False)
    wgh = nc.declare_dram_parameter("wgh", [OBS, E], bf16, isOutput=False)
    wgl = nc.declare_dram_parameter("wgl", [OBS, E], bf16, isOutput=False)
    bgd = nc.declare_dram_parameter("bg", [E, 1], f32, isOutput=False)
    wcat = nc.declare_dram_parameter("wcat", [E, 4, 128, 128], bf16, isOutput=False)
    bcat = nc.declare_dram_parameter("bcat", [E, 128], bf16, isOutput=False)
    trid = nc.declare_dram_parameter("tri", [128, 128], bf16, isOutput=False)
    identd = nc.declare_dram_parameter("ident", [16, 16], f32, isOutput=False)
    identbd = nc.declare_dram_parameter("identb", [16, 16], bf16, isOutput=False)
    iotad = nc.declare_dram_parameter("iota1p", [128, E], f32, isOutput=False)
    tvald = nc.declare_dram_parameter("tval", [128, NPT], f32, isOutput=False)
    ebased = nc.declare_dram_parameter("ebase", [1, E], f32, isOutput=False)
    onesd = nc.declare_dram_parameter("onesr", [1, 512], bf16, isOutput=False)
    ocold = nc.declare_dram_parameter("ocol", [128, 2], bf16, isOutput=False)
    ocolfd = nc.declare_dram_parameter("ocolf", [1, 128], f32, isOutput=False)
    meand = nc.declare_dram_parameter("mean", [BL, ACT], f32, isOutput=True)
    lstdd = nc.declare_dram_parameter("lstd", [BL, ACT], f32, isOutput=True)

    with tile.TileContext(nc) as tc, contextlib.ExitStack() as ctx:
      try:
        pp = ctx.enter_context(tc.tile_pool(name="pp", bufs=1))
        sp = ctx.enter_context(tc.tile_pool(name="sp", bufs=2))
        sp1 = ctx.enter_context(tc.tile_pool(name="sp1", bufs=1))
        psA = ctx.enter_context(tc.tile_pool(name="psA", bufs=1, space="PSUM"))
        psB = ctx.enter_context(tc.tile_pool(name="psB", bufs=2, space="PSUM"))
        psC = ctx.enter_context(tc.tile_pool(name="psC", bufs=1, space="PSUM"))

        # ---- consts ----
        wgh_sb = pp.tile([128, 4, E], bf16)
        nc.sync.dma_start(out=wgh_sb[:], in_=wgh.rearrange("(c p) e -> p c e", p=128))
        wgl_sb = pp.tile([128, 4, E], bf16)
        nc.sync.dma_start(out=wgl_sb[:], in_=wgl.rearrange("(c p) e -> p c e", p=128))
        bg_sb = pp.tile([16, 1], f32)
        nc.sync.dma_start(out=bg_sb[:], in_=bgd[:])
        bcat_sb = pp.tile([1, E, 128], bf16)
        nc.sync.dma_start(out=bcat_sb[:], in_=bcat[:].unsqueeze(0))
        tri_sb = pp.tile([128, 128], bf16)
        nc.sync.dma_start(out=tri_sb[:], in_=trid[:])
        ident_sb = pp.tile([16, 16], f32)
        nc.sync.dma_start(out=ident_sb[:], in_=identd[:])
        identb_sb = pp.tile([16, 16], bf16)
        nc.sync.dma_start(out=identb_sb[:], in_=identbd[:])
        iota_sb = pp.tile([128, E], f32)
        nc.sync.dma_start(out=iota_sb[:], in_=iotad[:])
        tval_sb = pp.tile([128, NPT], f32)
        nc.sync.dma_start(out=tval_sb[:], in_=tvald[:])
        ebase_sb = pp.tile([1, E], f32)
        nc.sync.dma_start(out=ebase_sb[:], in_=ebased[:])
        ones_sb = pp.tile([1, 512], bf16)
        nc.sync.dma_start(out=ones_sb[:], in_=onesd[:])
        ocol_sb = pp.tile([128, 2], bf16)
        nc.sync.dma_start(out=ocol_sb[:], in_=ocold[:])
        ocolf_sb = pp.tile([1, 128], f32)
        nc.sync.dma_start(out=ocolf_sb[:], in_=ocolfd[:])

        def replicate16(t):
            for g in range(1, 8):
                nc.sync.dma_start(out=t[16 * g:16 * g + 16, :], in_=t[0:16, :])


        x_sb = pp.tile([128, NT, OBS], bf16)
        xr = xd.rearrange("(i q) o -> q i o", q=128)
        RT = RC // 128                  # tiles per router chunk

        _loop = tc.For_i(0, REPS, 1) if REPS > 1 else None
        if _loop is not None:
            _loop.__enter__()

        # ---- x load + split + router ----
        ltm_ps = psA.tile([128, NT, E], f32, tag="ltm")
        for rc in range(NRC):
            xsl = slice(rc * RT, (rc + 1) * RT)
            xf = sp1.tile([128, RT, OBS], f32, tag="xf32")
            nc.sync.dma_start(out=xf[:], in_=xr[:, xsl, :])
            nc.scalar.activation(x_sb[:, xsl, :], xf[:], act_t.Copy)
            xlo = sp.tile([128, RT, OBS], bf16, tag="xlo", bufs=1)
            nc.vector.tensor_tensor(xlo[:], xf[:], x_sb[:, xsl, :], alu.subtract)
            xth = sp.tile([128, 4, RC], bf16, tag="xth")
            xtl = sp.tile([128, 4, RC], bf16, tag="xtl", bufs=1)
            for t in range(RT):
                tc_sl = slice(t * 128, (t + 1) * 128)
                nc.sync.dma_start_transpose(xth[:, :, tc_sl], x_sb[:, rc * RT + t, :])
                nc.sync.dma_start_transpose(xtl[:, :, tc_sl], xlo[:, t, :])
            for ns in range(RC // 512):
                ps = psB.tile([16, 512], f32, tag="mm")
                msl = slice(ns * 512, (ns + 1) * 512)
                for c in range(4):
                    nc.tensor.matmul(
                        ps[:], wgh_sb[:, c, :], xth[:, c, msl],
                        start=(c == 0), stop=False,
                    )
                    nc.tensor.matmul(
                        ps[:], wgl_sb[:, c, :], xth[:, c, msl],
                        start=False, stop=False,
                    )
                    nc.tensor.matmul(
                        ps[:], wgh_sb[:, c, :], xtl[:, c, msl],
                        start=False, stop=(c == 3),
                    )
                lt = sp.tile([16, 512], f32, tag="lt")
                nc.vector.tensor_scalar_add(lt[:], ps[:], bg_sb[:, 0:1])
                for ti in range(4):
                    gi = rc * RT + ns * 4 + ti
                    nc.tensor.transpose(
                        ltm_ps[:, gi, :], lt[:, ti * 128:(ti + 1) * 128], ident_sb[:]
                    )
        ltm = pp.tile([128, NT, E], f32, tag="ltm_sb")
        nc.vector.tensor_copy(ltm[:], ltm_ps[:])
        if STOP == "router":
            if _loop is not None:
                _loop.__exit__(None, None, None)
            raise _StopBuild

        # ---- softmax + top2 ----
        sm = pp.tile([128, NT, E], f32, tag="invA")
        ge = pp.tile([128, NT, E], f32, tag="invB")
        tmp = pp.tile([128, NT, E], f32, tag="smt")
        m1 = pp.tile([128, NT, 1], f32)
        Z = pp.tile([128, NT, 1], f32)
        g1 = pp.tile([128, NT, 1], f32)
        g2 = pp.tile([128, NT, 1], f32)
        av = pp.tile([128, NT, 1], f32)
        m2 = pp.tile([128, NT, 1], f32)
        oh_all = pp.tile([128, NPT, E], bf16)
        g_all = pp.tile([128, NPT], f32)

        nc.vector.tensor_reduce(m1[:], ltm[:], AX, alu.max)
        m1b = m1[:].broadcast_to((128, NT, E))
        iob = iota_sb[:].unsqueeze(1).broadcast_to((128, NT, E))
        nc.vector.tensor_tensor(sm[:], ltm[:], m1b, alu.subtract)
        nc.scalar.activation(sm[:], sm[:], act_t.Exp)
        nc.vector.tensor_reduce(Z[:], sm[:], AX, alu.add)
        nc.vector.reciprocal(g1[:], Z[:])
        nc.vector.tensor_tensor(ge[:], ltm[:], m1b, alu.is_ge)
        nc.vector.tensor_tensor(tmp[:], ge[:], iob, alu.mult)
        nc.vector.tensor_reduce(av[:], tmp[:], AX, alu.max)
        nc.vector.tensor_tensor(
            ge[:], iob, av[:].broadcast_to((128, NT, E)), alu.is_equal
        )
        nc.vector.tensor_copy(oh_all[:, 0:NT, :], ge[:])
        nc.vector.tensor_scalar_mul(tmp[:], ge[:], -1e30)
        nc.vector.tensor_tensor(tmp[:], ltm[:], tmp[:], alu.add)
        nc.vector.tensor_reduce(m2[:], tmp[:], AX, alu.max)
        nc.vector.tensor_tensor(
            ge[:], tmp[:], m2[:].broadcast_to((128, NT, E)), alu.is_ge
        )
        nc.vector.tensor_tensor(tmp[:], ge[:], iob, alu.mult)
        nc.vector.tensor_reduce(av[:], tmp[:], AX, alu.max)
        nc.vector.tensor_tensor(
            ge[:], iob, av[:].broadcast_to((128, NT, E)), alu.is_equal
        )
        nc.vector.tensor_copy(oh_all[:, NT:NPT, :], ge[:])
        nc.vector.tensor_tensor(m2[:], m2[:], m1[:], alu.subtract)
        nc.scalar.activation(m2[:], m2[:], act_t.Exp)
        nc.vector.tensor_tensor(g2[:], m2[:], g1[:], alu.mult)
        nc.vector.tensor_copy(g_all[:, 0:NT], g1[:, :, 0])
        nc.vector.tensor_copy(g_all[:, NT:NPT], g2[:, :, 0])

        # ---- counting sort ----
        totT_ps = psC.tile([16, NPT], f32, tag="small")
        for i in range(NPT):
            nc.tensor.matmul(
                totT_ps[:, i:i + 1], oh_all[:, i, :], ocol_sb[:, 0:1],
                start=True, stop=True,
            )
        totT_sb = pp.tile([16, NPT], bf16)
        nc.vector.tensor_copy(totT_sb[:], totT_ps[:])
        tot_ps = psC.tile([NPT, 16], bf16, tag="small2")
        nc.tensor.transpose(tot_ps[:], totT_sb[:], identb_sb[:])
        tot_sb = pp.tile([NPT, 16], bf16)
        nc.vector.tensor_copy(tot_sb[:], tot_ps[:])
        base_ps = psC.tile([NPT, 16], f32, tag="small3")
        nc.tensor.matmul(
            base_ps[:], tri_sb[0:NPT, 0:NPT], tot_sb[:], start=True, stop=False
        )
        nc.tensor.matmul(
            base_ps[:], ocolf_sb[:, 0:NPT], ebase_sb[:], start=False, stop=True
        )
        base_sb = pp.tile([NPT, 16], f32)
        nc.vector.tensor_copy(base_sb[:], base_ps[:])

        dst_f = pp.tile([128, NPT], f32)
        ohf = oh_all[:].rearrange("p i e -> p (i e)")
        NH = EW // SW
        IPH = SW // E                 # pair tiles per excl slice
        for h in range(NH):
            sl = slice(h * SW, (h + 1) * SW)
            brow = sp.tile([1, SW], f32, tag="brow")
            nc.sync.dma_start(
                out=brow[:].rearrange("a (i e) -> a i e", i=IPH),
                in_=base_sb[h * IPH:(h + 1) * IPH, :],
            )
            excl_ps = psA.tile([128, SW], f32, tag="excl")
            nc.tensor.matmul(excl_ps[:], tri_sb[:], ohf[:, sl], start=True, stop=False)
            nc.tensor.matmul(excl_ps[:], ocolf_sb[:], brow[:], start=False, stop=True)
            dmul = pp.tile([128, SW], f32, tag="smt")
            nc.vector.tensor_tensor(dmul[:], excl_ps[:], ohf[:, sl], alu.mult)
            nc.vector.tensor_reduce(
                dst_f[:, h * IPH:(h + 1) * IPH],
                dmul[:].rearrange("p (i e) -> p i e", e=E), AX, alu.add,
            )
        if STOP == "sort":
            if _loop is not None:
                _loop.__exit__(None, None, None)
            raise _StopBuild
        dst_i = pp.tile([128, NPT], i16)
        nc.vector.tensor_copy(dst_i[:], dst_f[:])
        dstw = pp.tile([128, NPAIR // 16], i16, tag="ltm_sb")
        nc.gpsimd.memset(dstw[:], 0)
        dwv = dstw[:].rearrange("p (i d) -> p i d", d=8)
        for d in range(8):
            nc.sync.dma_start(out=dwv[0:16, :, d], in_=dst_i[d * 16:(d + 1) * 16, :])
        replicate16(dstw)

        # ---- inversion scatter ----
        inv_own = pp.tile([128, NGRP, 64], f32, tag="invA")
        inv_peer = pp.tile([128, NGRP, 64], f32, tag="invB")
        nc.gpsimd.memset(inv_own[:], 0.0)
        nc.gpsimd.memset(inv_peer[:], 0.0)
        SCT = SCH // 128
        for s in range(NPAIR // SCH):
            insb = sp.tile([128, SCT, 64], f32, tag="insb")
            nc.gpsimd.memset(insb[:], 0.0)
            csl = slice(s * SCT, (s + 1) * SCT)
            nc.vector.tensor_copy(insb[:, :, 0], tval_sb[:, csl])
            nc.vector.tensor_copy(insb[:, :, 1], g_all[:, csl])
            nc.gpsimd.dma_scatter_add(
                inv_own[:], insb[:], dstw[:, s * (SCH // 16):(s + 1) * (SCH // 16)],
                SCH, SCH, 64,
                sbuf_tokens_per_rank=128, parity_reg=0, out_ap_other=inv_peer[:],
            )
        invv = pp.tile([128, NSLOT], f32)
        g_pos = pp.tile([128, NSLOT], f32)
        iv = invv[:].rearrange("p (g t) -> p g t", t=2)
        gv = g_pos[:].rearrange("p (g t) -> p g t", t=2)
        nc.vector.tensor_copy(iv[:, :, 0], inv_own[:, :, 0])
        nc.vector.tensor_copy(iv[:, :, 1], inv_peer[:, :, 0])
        nc.vector.tensor_copy(gv[:, :, 0], inv_own[:, :, 1])
        nc.vector.tensor_copy(gv[:, :, 1], inv_peer[:, :, 1])
        inv0 = pp.tile([128, NSLOT], i16)
        nc.vector.tensor_scalar_add(inv0[:], invv[:], -1.0)
        invw = pp.tile([128, NPC], i16)
        nc.gpsimd.memset(invw[:], 0)
        iwv = invw[:].rearrange("p (s d) -> p s d", d=8)
        for d in range(8):
            nc.sync.dma_start(out=iwv[0:16, :, d], in_=inv0[d * 16:(d + 1) * 16, :])
        replicate16(invw)
        idx_x = pp.tile([128, NPC], i16)
        nc.vector.tensor_scalar(idx_x[:], invw[:], BL - 1, None, alu.bitwise_and)
        msk = pp.tile([128, NPC], i16, tag="smt")
        nc.vector.tensor_scalar(msk[:], invw[:], 0, None, alu.is_lt)
        idx_sc = pp.tile([128, NPC], i16, tag="invw")
        nc.gpsimd.iota(
            idx_sc[:].rearrange("p (a b) -> p a b", b=8), [[0, NPC // 8], [16, 8]],
            base=BL, channel_multiplier=1,
        )
        nc.vector.tensor_tensor(idx_sc[:], idx_sc[:], idx_x[:], alu.subtract)
        nc.vector.tensor_tensor(idx_sc[:], msk[:], idx_sc[:], alu.mult)
        nc.vector.tensor_tensor(idx_sc[:], idx_x[:], idx_sc[:], alu.add)

        if STOP == "inv":
            if _loop is not None:
                _loop.__exit__(None, None, None)
            raise _StopBuild
        # ---- combine accumulators (share inversion slots) ----
        out_own = pp.tile([128, OG, 128], bf16, tag="invA")
        out_peer = pp.tile([128, OG, 128], bf16, tag="invB")
        nc.gpsimd.memset(out_own[:], 0.0)
        nc.gpsimd.memset(out_peer[:], 0.0)

        # ---- per-expert tail (half-expert chunks) ----
        for e in range(E):
            we = sp.tile([128, 4, 128], bf16, tag="we")
            nc.sync.dma_start(out=we[:], in_=wcat[e].rearrange("c p m -> p c m"))
            for half in range(CAP // HC):
                hb = e * CAP + half * HC      # global position base
                isl = slice(hb // 16, (hb + HC) // 16)
                xs = sp.tile([128, 4, HC], bf16, tag="xth")
                nc.gpsimd.dma_gather(
                    xs[:], x_sb[:], idx_x[:, isl], HC, HC, OBS,
                    transpose=True, sbuf_tokens_per_rank=128,
                    sbuf_free_dim_per_rank=OBS * 2,
                )
                yb = sp.tile([128, HC], bf16, tag="yb")
                for bi in range((HC + 511) // 512):
                    lo = bi * 512
                    hi = min(lo + 512, HC)
                    yps = psB.tile([128, 512], f32, tag="mm")
                    for c in range(4):
                        nc.tensor.matmul(
                            yps[:, 0:hi - lo], we[:, c, :], xs[:, c, lo:hi],
                            start=(c == 0), stop=False,
                        )
                    nc.tensor.matmul(
                        yps[:, 0:hi - lo], bcat_sb[:, e, :], ones_sb[:, 0:hi - lo],
                        start=False, stop=True,
                    )
                    if e % 2 == 0:
                        nc.scalar.activation(
                            yb[:, lo:hi], yps[:, 0:hi - lo], act_t.Copy
                        )
                    else:
                        nc.vector.tensor_copy(yb[:, lo:hi], yps[:, 0:hi - lo])
                ybr = sp.tile([128, HC // 128, 128], bf16, tag="ybr")
                nc.sync.dma_start_transpose(ybr[:], yb[:])
                nc.vector.tensor_tensor(
                    ybr[:], ybr[:],
                    g_pos[:, hb // 128:(hb + HC) // 128]
                    .unsqueeze(2).broadcast_to((128, HC // 128, 128)),
                    alu.mult,
                )
                nc.gpsimd.dma_scatter_add(
                    out_own[:], ybr[:], idx_sc[:, isl], HC, HC, 128,
                    sbuf_tokens_per_rank=128, parity_reg=0,
                    out_ap_other=out_peer[:],
                )

        if STOP == "tail":
            if _loop is not None:
                _loop.__exit__(None, None, None)
            raise _StopBuild
        # ---- finalize ----
        a = 0.5 * (LOG_STD_MAX - LOG_STD_MIN)
        bb = LOG_STD_MIN + a
        for h in range(FR):
            gsl = slice(h * (HT // 2), (h + 1) * (HT // 2))
            mf = sp.tile([128, HT, ACT], f32, tag="mf")
            sf = sp.tile([128, HT, ACT], f32, tag="sf")
            mfv = mf[:].rearrange("p (i t) a -> p i t a", t=2)
            sfv = sf[:].rearrange("p (i t) a -> p i t a", t=2)
            nc.scalar.activation(mfv[:, :, 0, :], out_own[:, gsl, 0:ACT], act_t.Copy)
            nc.scalar.activation(mfv[:, :, 1, :], out_peer[:, gsl, 0:ACT], act_t.Copy)
            nc.scalar.activation(sfv[:, :, 0, :], out_own[:, gsl, ACT:128], act_t.Tanh)
            nc.scalar.activation(sfv[:, :, 1, :], out_peer[:, gsl, ACT:128], act_t.Tanh)
            nc.vector.tensor_scalar(sf[:], sf[:], a, bb, alu.mult, alu.add)
            tsl = slice(h * HT, (h + 1) * HT)
            nc.sync.dma_start(
                out=meand.rearrange("(i q) a -> q i a", q=128)[:, tsl, :], in_=mf[:]
            )
            nc.sync.dma_start(
                out=lstdd.rearrange("(i q) a -> q i a", q=128)[:, tsl, :], in_=sf[:]
            )
        if _loop is not None:
            _loop.__exit__(None, None, None)
      except _StopBuild:
        pass

    nc.finalize()
    return nc


def host_inputs(Wg, bg, Wm, bm, Ws, bs, BL, CAP):
    import jax.numpy as jnp

    def to_bf16(arr):
        return np.asarray(jnp.asarray(np.asarray(arr, np.float32), jnp.bfloat16))

    NT = BL // 128
    NPT = 2 * NT
    wgh = to_bf16(Wg.T)
    wgl = to_bf16(Wg.T - wgh.astype(np.float32))
    Wfull = np.concatenate([Wm, Ws], axis=1)              # [E, 128, 512]
    wcat = np.ascontiguousarray(
        np.transpose(Wfull.reshape(E, 128, 4, 128), (0, 2, 3, 1))
    )                                                     # [E, 4, 128p, 128m]
    wcat = to_bf16(wcat)
    bcat = to_bf16(np.concatenate([bm, bs], axis=1))
    tri = to_bf16(np.triu(np.ones((128, 128), np.float32), 1))
    ident = np.eye(16, dtype=np.float32)
    identb = to_bf16(ident)
    iota1p = np.tile(np.arange(1, E + 1, dtype=np.float32), (128, 1))
    q = np.arange(128)[:, None]
    i = np.arange(NPT)[None, :]
    tval = (i * 128 + q + 1).astype(np.float32)
    ebase = (np.arange(E, dtype=np.float32) * CAP)[None, :]
    onesr = to_bf16(np.ones((1, 512), np.float32))
    ocol = to_bf16(np.ones((128, 2), np.float32))
    ocolf = np.ones((1, 128), np.float32)
    return {
        "wgh": wgh, "wgl": wgl, "bg": np.asarray(bg, np.float32).reshape(E, 1),
        "wcat": wcat, "bcat": bcat, "tri": tri, "ident": ident, "identb": identb,
        "iota1p": iota1p, "tval": tval, "ebase": ebase, "onesr": onesr,
        "ocol": ocol, "ocolf": ocolf,
    }


def kernel(x, Wg, bg, Wm, bm, Ws, bs, training):
    import concourse.bacc as bacc
    import concourse.mybir as mybir
    from concourse import tile
    from concourse.bass_utils import run_bass_kernel_spmd

    key = ("nc", BL, CAP)
    if key not in _COMPILED:
        nc = bacc.Bacc("TRN2", target_bir_lowering=False, debug=False)
        build(nc, mybir, tile, BL, CAP)
        _COMPILED[key] = nc
    nc = _COMPILED[key]

    x = np.asarray(x, np.float32)
    shared = host_inputs(
        np.asarray(Wg, np.float32), np.asarray(bg, np.float32),
        np.asarray(Wm, np.float32), np.asarray(bm, np.float32),
        np.asarray(Ws, np.float32), np.asarray(bs, np.float32), BL, CAP,
    )
    in_maps = [dict(shared, x=x[c * BL:(c + 1) * BL]) for c in range(NCORES)]
    res = run_bass_kernel_spmd(nc, in_maps, core_ids=list(range(NCORES)))
    mean = np.concatenate([res.results[c]["mean"] for c in range(NCORES)], axis=0)
    lstd = np.concatenate([res.results[c]["lstd"] for c in range(NCORES)], axis=0)
    return mean, lstd

